# revision 25
# baseline (speedup 1.0000x reference)
"""Causal scaled-dot-product attention for Trainium2 (Bass/Tile), 8-core SPMD.

Problem: B=2, H=16, S=2048, D=128 fp32, causal mask, softmax(QK^T/sqrt(D)) @ V.
Sharding: batch*heads (32) split across 8 cores, 4 heads per core. Attention is
independent per (b,h): no communication.

Host-side prep (part of the sharding step in kernel()): Q and K are
transposed to [d, s] and all of Q^T, K^T, V are cast to bf16 before upload.
The device kernel is then pure compute -- no on-device transposes or casts,
and half the input DMA volume (6 MB/core instead of 12).

Per-head algorithm (S^T layout -- avoids any transpose of the probability
matrix):
  - for each 512-wide query chunk c:
      for each key tile j (128 keys) at or below the diagonal:
        S^T[j] = K_j @ Q_c^T          (bf16 matmul, fp32 PSUM)
        P^T[j] = exp(S^T[j] / temp)   (ACT, PSUM->SBUF, bf16)
        diagonal tiles masked with an upper-triangular constant
        OUT^T  += V_j^T @ P^T[j]      (bf16 matmul, V in natural layout)
        den    += ones^T @ P^T[j]     (bf16 matmul, [1, 512])
      tail: rcq = 1/den broadcast across partitions (gpsimd); OUT^T * rcq;
            4 bf16 PE transposes -> [q, d]; cast-copy to f32 -> DRAM
Softmax max-subtraction is skipped: logits are bounded (~20) so exp is safe in
fp32, and softmax is shift-invariant.

Emission is software-pipelined so the in-order PE never waits: PV/den matmuls
for group g are emitted after group g+1's QK/exp; chunk tails are deferred;
the next head's loads are issued a full head ahead. A burst of dummy matmuls
at kernel start ramps the PE clock (HAM) out of its cold 1.2 GHz state while
the first DMAs land.
"""
import numpy as np
import ml_dtypes

import concourse.bacc as bacc
import concourse.tile as tile
import concourse.mybir as mybir
from concourse.bass_utils import run_bass_kernel_spmd
from concourse.masks import make_identity, make_upper_triangular

F32 = mybir.dt.float32
BF16 = mybir.dt.bfloat16
EXP = mybir.ActivationFunctionType.Exp

B, H, S, D = 2, 16, 2048, 128
TEMPERATURE = 11.313708498984761  # sqrt(128)
N_CORES = 8
HEADS_PER_CORE = (B * H) // N_CORES  # 4
P = 128                    # partitions / tile edge
CHUNK = 512                # query chunk (1 PSUM bank of fp32)
N_KT = S // P              # 16 key tiles per head
N_CH = S // CHUNK          # 4 query chunks per head
N_WARMUP = 26              # dummy matmuls to ramp the PE clock at start


def build_attention_nc(rep=1):
    nc = bacc.Bacc("TRN2", target_bir_lowering=False, debug=False,
                   num_devices=N_CORES)
    qt_d = nc.dram_tensor("qt", [HEADS_PER_CORE, D, S], BF16, kind="ExternalInput").ap()
    kt_d = nc.dram_tensor("kt", [HEADS_PER_CORE, D, S], BF16, kind="ExternalInput").ap()
    v_d = nc.dram_tensor("v", [HEADS_PER_CORE, S, D], BF16, kind="ExternalInput").ap()
    o_d = nc.dram_tensor("out", [HEADS_PER_CORE, S, D], F32, kind="ExternalOutput").ap()

    n_heads = rep * HEADS_PER_CORE

    with tile.TileContext(nc) as tc:
        with tc.tile_pool(name="consts", bufs=1) as consts, \
             tc.tile_pool(name="qkt", bufs=2) as qkt, \
             tc.tile_pool(name="px", bufs=6) as px, \
             tc.tile_pool(name="sm", bufs=4) as sm, \
             tc.tile_pool(name="ps_s", bufs=2, space="PSUM") as ps_s, \
             tc.tile_pool(name="ps_o", bufs=2, space="PSUM") as ps_o, \
             tc.tile_pool(name="ps_d", bufs=1, space="PSUM") as ps_d, \
             tc.tile_pool(name="ps_t", bufs=1, space="PSUM") as ps_t:

            # ---- constants ----
            utm = consts.tile([P, P], BF16)  # utm[k,q] = 1 iff q >= k
            make_upper_triangular(nc, utm, val=1.0, diag=True)
            ident_h = consts.tile([P, P], BF16)
            make_identity(nc, ident_h)
            ones_col = consts.tile([P, 1], BF16)
            nc.vector.memset(ones_col, 1.0)

            # ---- PE clock warmup: dummy bf16 matmuls while DMAs land ----
            for w in range(N_WARMUP):
                pw = ps_o.tile([P, CHUNK], F32, tag="po", name="po")
                nc.tensor.matmul(pw[:, 0:P], utm, utm, start=True, stop=True,
                                 skip_group_check=True)

            head_state = {}

            def emit_load(hh, first=False):
                h = hh % HEADS_PER_CORE
                qT = qkt.tile([P, S], BF16, tag="qT", name="qT")
                kT = qkt.tile([P, S], BF16, tag="kT", name="kT")
                vnr = qkt.tile([P, N_KT, P], BF16, tag="vnr", name="vnr")
                head_state[hh] = dict(qT=qT, kT=kT, vnr=vnr)
                if first:
                    # k first (first matmuls need it), in halves so the first
                    # chunk can start after ~1 us of DMA
                    h8 = N_KT // 2
                    nc.sync.dma_start(out=kT[:, 0:S // 2],
                                      in_=kt_d[h, :, 0:S // 2])
                    nc.sync.dma_start(out=qT[:, 0:S // 2],
                                      in_=qt_d[h, :, 0:S // 2])
                    nc.sync.dma_start(
                        out=vnr[:, 0:h8],
                        in_=v_d[h, 0:S // 2].rearrange("(t p) d -> p t d", p=P))
                    nc.sync.dma_start(out=kT[:, S // 2:S],
                                      in_=kt_d[h, :, S // 2:S])
                    nc.sync.dma_start(out=qT[:, S // 2:S],
                                      in_=qt_d[h, :, S // 2:S])
                    nc.sync.dma_start(
                        out=vnr[:, h8:N_KT],
                        in_=v_d[h, S // 2:S].rearrange("(t p) d -> p t d", p=P))
                else:
                    nc.sync.dma_start(out=kT, in_=kt_d[h])
                    nc.sync.dma_start(out=qT, in_=qt_d[h])
                    nc.sync.dma_start(
                        out=vnr, in_=v_d[h].rearrange("(t p) d -> p t d", p=P))

            def make_pv(st, offs, pexp, psum_o, psum_d, jmax):
                def emit():
                    for (j, oj, base) in offs:
                        nc.tensor.matmul(
                            psum_o[:, oj:CHUNK], st["vnr"][:, j, :],
                            pexp[:, base + oj:base + CHUNK],
                            start=(j == 0), stop=(j == jmax),
                            skip_group_check=True)
                        nc.tensor.matmul(
                            psum_d[:, oj:CHUNK], ones_col,
                            pexp[:, base + oj:base + CHUNK],
                            start=(j == 0), stop=(j == jmax),
                            skip_group_check=True)
                return emit

            def make_tail(hh, c, psum_o, psum_d):
                def emit():
                    h = hh % HEADS_PER_CORE
                    # denominators out of PSUM first (releases the den bank),
                    # then reciprocal once per chunk
                    dens = sm.tile([1, CHUNK], F32, tag="dens", name="dens")
                    nc.vector.tensor_copy(dens, psum_d)
                    rcq = sm.tile([1, CHUNK], F32, tag="rcq", name="rcq")
                    nc.vector.reciprocal_approx_fast(rcq, dens)
                    # physically replicate 1/den across partitions (gpsimd),
                    # then normalize OUT^T columns with one elementwise mul
                    rcb = sm.tile([P, CHUNK], F32, tag="rcb", name="rcb")
                    nc.gpsimd.partition_broadcast(rcb, rcq)
                    outn = sm.tile([P, CHUNK], BF16, tag="outn", name="outn")
                    nc.vector.tensor_mul(outn, psum_o, rcb)
                    # transpose OUT^T back to [q, d] (bf16, cheap)
                    ptr2 = ps_t.tile([P, CHUNK], BF16, tag="ptr", name="ptr")
                    for tt in range(4):
                        nc.tensor.transpose(
                            ptr2[:, tt * P:(tt + 1) * P],
                            outn[:, tt * P:(tt + 1) * P], ident_h)
                    # evacuate + widen to f32 for the store
                    outT = sm.tile([P, 4, P], F32, tag="outT", name="outT")
                    nc.vector.tensor_copy(
                        outT, ptr2.rearrange("p (a b) -> p a b", b=P))
                    nc.sync.dma_start(
                        out=o_d[h, CHUNK * c:CHUNK * (c + 1), :].rearrange(
                            "(t p) d -> p t d", p=P),
                        in_=outT)
                return emit

            emit_load(0, first=True)

            for hh in range(n_heads):
                st = head_state[hh]
                if hh + 1 < n_heads:
                    emit_load(hh + 1)

                pending_pv = None          # PV/den of previous group
                deferred = []              # [(age, closure)] chunk tails
                group_idx = 0

                for c in range(N_CH):
                    jmax = 4 * c + 3
                    psum_o = ps_o.tile([P, CHUNK], F32, tag="po", name="po")
                    psum_d = ps_d.tile([1, CHUNK], F32, tag="pd", name="pd")

                    for jp in range((jmax + 2) // 2):
                        j0 = 2 * jp
                        js = [j for j in (j0, j0 + 1) if j <= jmax]
                        psum_s = ps_s.tile([P, 2 * CHUNK], F32, tag="psm",
                                           name="psm")
                        pexp = px.tile([P, 2 * CHUNK], BF16, tag="pexp",
                                       name="pexp")

                        offs = []
                        for j in js:
                            oj = max(0, P * j - CHUNK * c)
                            base = (j - j0) * CHUNK
                            offs.append((j, oj, base))
                            # full-width write even on diagonal tiles: the
                            # below-diagonal columns hold bounded garbage that
                            # exp processes but PV/den never consume; this
                            # keeps every exp read covered by this tile's
                            # writes (no stale-PSUM reads)
                            nc.tensor.matmul(
                                psum_s[:, base:base + CHUNK],
                                st["kT"][:, j * P:(j + 1) * P],
                                st["qT"][:, CHUNK * c:CHUNK * (c + 1)],
                                start=True, stop=True)

                        # exp (+ causal masking of diagonal 128-blocks,
                        # applied in place after the exp). Diagonal pairs are
                        # exp'd in a single wide op spanning both j regions;
                        # the dead gap between them is never consumed.
                        diag = any(j * P >= CHUNK * c for (j, oj, base) in offs)
                        lo = offs[0][2] + offs[0][1]
                        hi = offs[-1][2] + CHUNK
                        nc.scalar.activation(
                            pexp[:, lo:hi], psum_s[:, lo:hi],
                            EXP, scale=1.0 / TEMPERATURE)
                        if diag:
                            for (j, oj, base) in offs:
                                if j * P >= CHUNK * c:
                                    nc.vector.tensor_mul(
                                        pexp[:, base + oj:base + oj + P],
                                        pexp[:, base + oj:base + oj + P], utm)

                        if pending_pv is not None:
                            pending_pv()
                        pending_pv = make_pv(st, offs, pexp, psum_o, psum_d,
                                             jmax)

                        group_idx += 1
                        tail_age = 1 if hh == n_heads - 1 else 2
                        for item in list(deferred):
                            if group_idx - item[0] >= tail_age:
                                item[1]()
                                deferred.remove(item)

                    deferred.append((group_idx, make_tail(hh, c, psum_o,
                                                          psum_d)))

                # flush this head
                if pending_pv is not None:
                    pending_pv()
                for item in deferred:
                    item[1]()

    nc.compile()
    return nc


_NC_CACHE = None


def _get_nc():
    global _NC_CACHE
    if _NC_CACHE is None:
        _NC_CACHE = build_attention_nc()
    return _NC_CACHE


def kernel(q, k, v, mask=None, _trace=False):
    """Full-input entry point: q,k,v [2,16,2048,128] f32, mask [2,1,2048,2048]
    int32 (causal; the kernel hardcodes causality and does not read it).
    Returns [2,16,2048,128] f32. Q/K are transposed and all inputs cast to
    bf16 host-side as part of the sharding step."""
    nc = _get_nc()
    BH = B * H
    qt = np.ascontiguousarray(
        np.asarray(q, dtype=np.float32).reshape(BH, S, D).transpose(0, 2, 1)
    ).astype(ml_dtypes.bfloat16)
    kt = np.ascontiguousarray(
        np.asarray(k, dtype=np.float32).reshape(BH, S, D).transpose(0, 2, 1)
    ).astype(ml_dtypes.bfloat16)
    vf = np.asarray(v, dtype=np.float32).reshape(BH, S, D).astype(
        ml_dtypes.bfloat16)
    in_maps = []
    for i in range(N_CORES):
        sl = slice(i * HEADS_PER_CORE, (i + 1) * HEADS_PER_CORE)
        in_maps.append({"qt": qt[sl], "kt": kt[sl], "v": vf[sl]})
    res = run_bass_kernel_spmd(nc, in_maps, list(range(N_CORES)), trace=_trace)
    out = np.concatenate([res.results[i]["out"] for i in range(N_CORES)], axis=0)
    out = out.reshape(B, H, S, D).astype(np.float32)
    if _trace:
        return out, res
    return out


# revision 26
# speedup vs baseline: 1.4354x; 1.4354x over previous
"""Causal scaled-dot-product attention for Trainium2 (Bass/Tile), 8-core SPMD.

Problem: B=2, H=16, S=2048, D=128 fp32, causal mask, softmax(QK^T/sqrt(D)) @ V.
Sharding: batch*heads (32) split across 8 cores, 4 heads per core. Attention is
independent per (b,h): no communication.

Host-side prep (part of the sharding step in kernel()): Q and K are
transposed to [d, s] and all of Q^T, K^T, V are cast to bf16 before upload.
The device kernel is then pure compute -- no on-device transposes or casts,
and half the input DMA volume (6 MB/core instead of 12).

Per-head algorithm (S^T layout -- avoids any transpose of the probability
matrix):
  - for each 512-wide query chunk c:
      for each key tile j (128 keys) at or below the diagonal:
        S^T[j] = K_j @ Q_c^T          (bf16 matmul, fp32 PSUM)
        P^T[j] = exp(S^T[j] / temp)   (ACT, PSUM->SBUF, bf16)
        diagonal tiles masked with an upper-triangular constant
        OUT^T  += V_j^T @ P^T[j]      (bf16 matmul, V in natural layout)
        den    += ones^T @ P^T[j]     (bf16 matmul, [1, 512])
      tail: rcq = 1/den broadcast across partitions (gpsimd); OUT^T * rcq;
            4 bf16 PE transposes -> [q, d]; cast-copy to f32 -> DRAM
Softmax max-subtraction is skipped: logits are bounded (~20) so exp is safe in
fp32, and softmax is shift-invariant.

Emission is software-pipelined so the in-order PE never waits: PV/den matmuls
for group g are emitted after group g+1's QK/exp; chunk tails are deferred;
the next head's loads are issued a full head ahead. A burst of dummy matmuls
at kernel start ramps the PE clock (HAM) out of its cold 1.2 GHz state while
the first DMAs land.
"""
import numpy as np
import ml_dtypes

import concourse.bacc as bacc
import concourse.tile as tile
import concourse.mybir as mybir
from concourse.bass_utils import run_bass_kernel_spmd
from concourse.masks import make_identity, make_upper_triangular

F32 = mybir.dt.float32
BF16 = mybir.dt.bfloat16
FP8 = mybir.dt.float8e4
EXP = mybir.ActivationFunctionType.Exp

B, H, S, D = 2, 16, 2048, 128
TEMPERATURE = 11.313708498984761  # sqrt(128)
N_CORES = 8
HEADS_PER_CORE = (B * H) // N_CORES  # 4
P = 128                    # partitions / tile edge
CHUNK = 512                # query chunk (1 PSUM bank of fp32)
N_KT = S // P              # 16 key tiles per head
N_CH = S // CHUNK          # 4 query chunks per head
N_WARMUP = 26              # dummy matmuls to ramp the PE clock at start


def build_attention_nc(rep=1):
    nc = bacc.Bacc("TRN2", target_bir_lowering=False, debug=False,
                   num_devices=N_CORES)
    qt_d = nc.dram_tensor("qt", [HEADS_PER_CORE, D, S], BF16, kind="ExternalInput").ap()
    kt_d = nc.dram_tensor("kt", [HEADS_PER_CORE, D, S], BF16, kind="ExternalInput").ap()
    v_d = nc.dram_tensor("v", [HEADS_PER_CORE, S, D], BF16, kind="ExternalInput").ap()
    v8_d = nc.dram_tensor("v8", [HEADS_PER_CORE, S, D], FP8, kind="ExternalInput").ap()
    o_d = nc.dram_tensor("out", [HEADS_PER_CORE, S, D], F32, kind="ExternalOutput").ap()

    n_heads = rep * HEADS_PER_CORE

    with tile.TileContext(nc) as tc:
        with tc.tile_pool(name="consts", bufs=1) as consts, \
             tc.tile_pool(name="qkt", bufs=2) as qkt, \
             tc.tile_pool(name="px", bufs=6) as px, \
             tc.tile_pool(name="sm", bufs=4) as sm, \
             tc.tile_pool(name="ps_s", bufs=2, space="PSUM") as ps_s, \
             tc.tile_pool(name="ps_o", bufs=2, space="PSUM") as ps_o, \
             tc.tile_pool(name="ps_d", bufs=1, space="PSUM") as ps_d, \
             tc.tile_pool(name="ps_t", bufs=1, space="PSUM") as ps_t:

            # ---- constants ----
            utm = consts.tile([P, P], BF16)  # utm[k,q] = 1 iff q >= k
            make_upper_triangular(nc, utm, val=1.0, diag=True)
            ident_h = consts.tile([P, P], BF16)
            make_identity(nc, ident_h)
            ones_col = consts.tile([P, 1], BF16)
            nc.vector.memset(ones_col, 1.0)
            # fp8 ones for DoubleRow den matmuls; the pair dim is sliced out
            # of a padded tile so its byte-step is 16 (ISA requirement)
            ones8p = consts.tile([P, 2, 16], FP8)
            nc.vector.memset(ones8p, 1.0)
            nbias = consts.tile([P, 1], F32)
            nc.vector.memset(nbias, -2.0)

            # ---- PE clock warmup: dummy bf16 matmuls while DMAs land ----
            for w in range(N_WARMUP):
                pw = ps_o.tile([P, CHUNK], F32, tag="po", name="po")
                nc.tensor.matmul(pw[:, 0:P], utm, utm, start=True, stop=True,
                                 skip_group_check=True)

            head_state = {}

            def emit_load(hh, first=False):
                h = hh % HEADS_PER_CORE
                qT = qkt.tile([P, S], BF16, tag="qT", name="qT")
                kT = qkt.tile([P, S], BF16, tag="kT", name="kT")
                vnr = qkt.tile([P, N_KT, P], BF16, tag="vnr", name="vnr")
                v8 = qkt.tile([P, N_KT, P], FP8, tag="v8", name="v8")
                st = dict(qT=qT, kT=kT, vnr=vnr, v8=v8)
                head_state[hh] = st
                if first:
                    # k first (first matmuls need it), in halves so the first
                    # chunk can start after ~1 us of DMA
                    h8 = N_KT // 2
                    nc.sync.dma_start(out=kT[:, 0:S // 2],
                                      in_=kt_d[h, :, 0:S // 2])
                    nc.sync.dma_start(out=qT[:, 0:S // 2],
                                      in_=qt_d[h, :, 0:S // 2])
                    nc.sync.dma_start(
                        out=vnr[:, 0:h8],
                        in_=v_d[h, 0:S // 2].rearrange("(t p) d -> p t d", p=P))
                    nc.sync.dma_start(
                        out=st["v8"][:, 0:h8],
                        in_=v8_d[h, 0:S // 2].rearrange("(t p) d -> p t d", p=P))
                    nc.sync.dma_start(out=kT[:, S // 2:S],
                                      in_=kt_d[h, :, S // 2:S])
                    nc.sync.dma_start(out=qT[:, S // 2:S],
                                      in_=qt_d[h, :, S // 2:S])
                    nc.sync.dma_start(
                        out=vnr[:, h8:N_KT],
                        in_=v_d[h, S // 2:S].rearrange("(t p) d -> p t d", p=P))
                    nc.sync.dma_start(
                        out=st["v8"][:, h8:N_KT],
                        in_=v8_d[h, S // 2:S].rearrange("(t p) d -> p t d", p=P))
                else:
                    nc.sync.dma_start(out=kT, in_=kt_d[h])
                    nc.sync.dma_start(out=qT, in_=qt_d[h])
                    nc.sync.dma_start(
                        out=vnr, in_=v_d[h].rearrange("(t p) d -> p t d", p=P))
                    nc.sync.dma_start(
                        out=st["v8"], in_=v8_d[h].rearrange("(t p) d -> p t d", p=P))

            def make_pv(st, offs, pexp, psum_o, psum_d, jmax):
                def emit():
                    for (j, oj, base) in offs:
                        nc.tensor.matmul(
                            psum_o[:, oj:CHUNK], st["vnr"][:, j, :],
                            pexp[:, base + oj:base + CHUNK],
                            start=(j == 0), stop=(j == jmax),
                            skip_group_check=True)
                        nc.tensor.matmul(
                            psum_d[:, oj:CHUNK], ones_col,
                            pexp[:, base + oj:base + CHUNK],
                            start=(j == 0), stop=(j == jmax),
                            skip_group_check=True)
                return emit

            def make_pv_dr(st, js, pexp8, psum_o, psum_d, jmax):
                # fp8 DoubleRow: one matmul contracts both key tiles of the
                # pair (256 rows) at 0.5 cycles/row
                j0 = js[0]
                def emit():
                    pe3 = pexp8.rearrange("p (i n) -> p i n", i=2)
                    nc.tensor.matmul(
                        psum_o, st["v8"][:, j0:j0 + 2, :], pe3,
                        start=(j0 == 0), stop=(j0 + 1 == jmax),
                        perf_mode=mybir.MatmulPerfMode.DoubleRow,
                        skip_group_check=True)
                    nc.tensor.matmul(
                        psum_d, ones8p[:, :, 0:1], pe3,
                        start=(j0 == 0), stop=(j0 + 1 == jmax),
                        perf_mode=mybir.MatmulPerfMode.DoubleRow,
                        skip_group_check=True)
                return emit

            def make_tail(hh, c, psum_o, psum_d):
                def emit():
                    h = hh % HEADS_PER_CORE
                    # denominators out of PSUM first (releases the den bank),
                    # then reciprocal once per chunk
                    dens = sm.tile([1, CHUNK], F32, tag="dens", name="dens")
                    nc.vector.tensor_copy(dens, psum_d)
                    rcq = sm.tile([1, CHUNK], F32, tag="rcq", name="rcq")
                    nc.vector.reciprocal_approx_fast(rcq, dens)
                    # physically replicate 1/den across partitions (gpsimd),
                    # then normalize OUT^T columns with one elementwise mul
                    rcb = sm.tile([P, CHUNK], F32, tag="rcb", name="rcb")
                    nc.gpsimd.partition_broadcast(rcb, rcq)
                    outn = sm.tile([P, CHUNK], BF16, tag="outn", name="outn")
                    nc.vector.tensor_mul(outn, psum_o, rcb)
                    # transpose OUT^T back to [q, d] (bf16, cheap)
                    ptr2 = ps_t.tile([P, CHUNK], BF16, tag="ptr", name="ptr")
                    for tt in range(4):
                        nc.tensor.transpose(
                            ptr2[:, tt * P:(tt + 1) * P],
                            outn[:, tt * P:(tt + 1) * P], ident_h)
                    # evacuate + widen to f32 for the store
                    outT = sm.tile([P, 4, P], F32, tag="outT", name="outT")
                    nc.vector.tensor_copy(
                        outT, ptr2.rearrange("p (a b) -> p a b", b=P))
                    nc.sync.dma_start(
                        out=o_d[h, CHUNK * c:CHUNK * (c + 1), :].rearrange(
                            "(t p) d -> p t d", p=P),
                        in_=outT)
                return emit

            emit_load(0, first=True)

            for hh in range(n_heads):
                st = head_state[hh]
                if hh + 1 < n_heads:
                    emit_load(hh + 1)

                pending_pv = None          # PV/den of previous group
                deferred = []              # [(age, closure)] chunk tails
                group_idx = 0

                for c in range(N_CH):
                    jmax = 4 * c + 3
                    psum_o = ps_o.tile([P, CHUNK], F32, tag="po", name="po")
                    psum_d = ps_d.tile([1, CHUNK], F32, tag="pd", name="pd")

                    for jp in range((jmax + 2) // 2):
                        j0 = 2 * jp
                        js = [j for j in (j0, j0 + 1) if j <= jmax]
                        psum_s = ps_s.tile([P, 2 * CHUNK], F32, tag="psm",
                                           name="psm")
                        nondiag = (j0 + 1 <= jmax) and ((j0 + 1) * P < CHUNK * c)
                        pexp = px.tile([P, 2 * CHUNK], FP8 if nondiag else BF16,
                                       tag="pexp", name="pexp")

                        offs = []
                        for j in js:
                            oj = max(0, P * j - CHUNK * c)
                            base = (j - j0) * CHUNK
                            offs.append((j, oj, base))
                            # full-width write even on diagonal tiles: the
                            # below-diagonal columns hold bounded garbage that
                            # exp processes but PV/den never consume; this
                            # keeps every exp read covered by this tile's
                            # writes (no stale-PSUM reads)
                            nc.tensor.matmul(
                                psum_s[:, base:base + CHUNK],
                                st["kT"][:, j * P:(j + 1) * P],
                                st["qT"][:, CHUNK * c:CHUNK * (c + 1)],
                                start=True, stop=True)

                        # exp (+ causal masking of diagonal 128-blocks,
                        # applied in place after the exp). Diagonal pairs are
                        # exp'd in a single wide op spanning both j regions;
                        # the dead gap between them is never consumed.
                        diag = any(j * P >= CHUNK * c for (j, oj, base) in offs)
                        lo = offs[0][2] + offs[0][1]
                        hi = offs[-1][2] + CHUNK
                        # uniform logit shift keeps exp inside fp8e4m3
                        # range (softmax is shift-invariant; den and out
                        # scale by the same e^-2)
                        nc.scalar.activation(
                            pexp[:, lo:hi], psum_s[:, lo:hi],
                            EXP, scale=1.0 / TEMPERATURE, bias=nbias)
                        if diag:
                            for (j, oj, base) in offs:
                                if j * P >= CHUNK * c:
                                    nc.vector.tensor_mul(
                                        pexp[:, base + oj:base + oj + P],
                                        pexp[:, base + oj:base + oj + P], utm)

                        if pending_pv is not None:
                            pending_pv()
                        if nondiag:
                            pending_pv = make_pv_dr(st, js, pexp, psum_o,
                                                    psum_d, jmax)
                        else:
                            pending_pv = make_pv(st, offs, pexp, psum_o,
                                                 psum_d, jmax)

                        group_idx += 1
                        tail_age = 1 if hh == n_heads - 1 else 2
                        for item in list(deferred):
                            if group_idx - item[0] >= tail_age:
                                item[1]()
                                deferred.remove(item)

                    deferred.append((group_idx, make_tail(hh, c, psum_o,
                                                          psum_d)))

                # flush this head
                if pending_pv is not None:
                    pending_pv()
                for item in deferred:
                    item[1]()

    nc.compile()
    return nc


_NC_CACHE = None


def _get_nc():
    global _NC_CACHE
    if _NC_CACHE is None:
        _NC_CACHE = build_attention_nc()
    return _NC_CACHE


def kernel(q, k, v, mask=None, _trace=False):
    """Full-input entry point: q,k,v [2,16,2048,128] f32, mask [2,1,2048,2048]
    int32 (causal; the kernel hardcodes causality and does not read it).
    Returns [2,16,2048,128] f32. Q/K are transposed and all inputs cast to
    bf16 host-side as part of the sharding step."""
    nc = _get_nc()
    BH = B * H
    qt = np.ascontiguousarray(
        np.asarray(q, dtype=np.float32).reshape(BH, S, D).transpose(0, 2, 1)
    ).astype(ml_dtypes.bfloat16)
    kt = np.ascontiguousarray(
        np.asarray(k, dtype=np.float32).reshape(BH, S, D).transpose(0, 2, 1)
    ).astype(ml_dtypes.bfloat16)
    vf32 = np.asarray(v, dtype=np.float32).reshape(BH, S, D)
    vf = vf32.astype(ml_dtypes.bfloat16)
    v8 = vf32.astype(mybir.dt.np(mybir.dt.float8e4))
    in_maps = []
    for i in range(N_CORES):
        sl = slice(i * HEADS_PER_CORE, (i + 1) * HEADS_PER_CORE)
        in_maps.append({"qt": qt[sl], "kt": kt[sl], "v": vf[sl],
                        "v8": v8[sl]})
    res = run_bass_kernel_spmd(nc, in_maps, list(range(N_CORES)), trace=_trace)
    out = np.concatenate([res.results[i]["out"] for i in range(N_CORES)], axis=0)
    out = out.reshape(B, H, S, D).astype(np.float32)
    if _trace:
        return out, res
    return out


# revision 27
# speedup vs baseline: 1.4406x; 1.0037x over previous
"""Causal scaled-dot-product attention for Trainium2 (Bass/Tile), 8-core SPMD.

Problem: B=2, H=16, S=2048, D=128 fp32, causal mask, softmax(QK^T/sqrt(D)) @ V.
Sharding: batch*heads (32) split across 8 cores, 4 heads per core. Attention is
independent per (b,h): no communication.

Host-side prep (part of the sharding step in kernel()): Q and K are
transposed to [d, s] and cast to bf16; V is uploaded in both bf16 and
fp8e4m3. The device kernel is then pure compute -- no on-device transposes
or casts, and roughly half the input DMA volume.

Per-head algorithm (S^T layout -- avoids any transpose of the probability
matrix); key tiles are processed in pairs (j0, j1):
  - for each 512-wide query chunk c:
      for each key-tile pair at or below the diagonal:
        S^T[j0|j1] = K_j @ Q_c^T      (bf16 matmuls, fp32 PSUM)
        P^T = exp(S^T/temp - 2)       (ACT, PSUM->SBUF; fp8 for off-diagonal
                                       pairs, bf16 for diagonal pairs; the -2
                                       shift keeps exp inside fp8 range and
                                       cancels in the softmax)
        diagonal tiles masked with an upper-triangular constant (DVE)
        off-diagonal pairs: OUT^T += V8_pair @ P^T and den += ones8 @ P^T as
          single fp8 DoubleRow matmuls contracting both tiles (256 rows) at
          0.5 cycles/row; diagonal pairs stay bf16 per-tile (fp8 on the
          near-1 diagonal probabilities would breach the error budget)
      tail: rcq = 1/den broadcast across partitions (gpsimd); OUT^T * rcq;
            4 bf16 PE transposes -> [q, d]; cast-copy to f32 -> DRAM
Softmax max-subtraction is skipped: logits are bounded (~20) so exp is safe,
and softmax is shift-invariant.

Emission is software-pipelined so the in-order PE never waits: PV/den matmuls
for group g are emitted after group g+1's QK/exp; chunk tails are deferred;
the next head's loads are issued a full head ahead. A burst of dummy matmuls
at kernel start ramps the PE clock (HAM) out of its cold 1.2 GHz state while
the first DMAs land.
"""
import numpy as np
import ml_dtypes

import concourse.bacc as bacc
import concourse.tile as tile
import concourse.mybir as mybir
from concourse.bass_utils import run_bass_kernel_spmd
from concourse.masks import make_identity, make_upper_triangular

F32 = mybir.dt.float32
BF16 = mybir.dt.bfloat16
FP8 = mybir.dt.float8e4
EXP = mybir.ActivationFunctionType.Exp

B, H, S, D = 2, 16, 2048, 128
TEMPERATURE = 11.313708498984761  # sqrt(128)
N_CORES = 8
HEADS_PER_CORE = (B * H) // N_CORES  # 4
P = 128                    # partitions / tile edge
CHUNK = 512                # query chunk (1 PSUM bank of fp32)
N_KT = S // P              # 16 key tiles per head
N_CH = S // CHUNK          # 4 query chunks per head
N_WARMUP = 26              # dummy matmuls to ramp the PE clock at start


def build_attention_nc(rep=1):
    nc = bacc.Bacc("TRN2", target_bir_lowering=False, debug=False,
                   num_devices=N_CORES)
    qt_d = nc.dram_tensor("qt", [HEADS_PER_CORE, D, S], BF16, kind="ExternalInput").ap()
    kt_d = nc.dram_tensor("kt", [HEADS_PER_CORE, D, S], BF16, kind="ExternalInput").ap()
    v_d = nc.dram_tensor("v", [HEADS_PER_CORE, S, D], BF16, kind="ExternalInput").ap()
    v8_d = nc.dram_tensor("v8", [HEADS_PER_CORE, S, D], FP8, kind="ExternalInput").ap()
    o_d = nc.dram_tensor("out", [HEADS_PER_CORE, S, D], F32, kind="ExternalOutput").ap()

    n_heads = rep * HEADS_PER_CORE

    with tile.TileContext(nc) as tc:
        with tc.tile_pool(name="consts", bufs=1) as consts, \
             tc.tile_pool(name="qkt", bufs=2) as qkt, \
             tc.tile_pool(name="px", bufs=6) as px, \
             tc.tile_pool(name="sm", bufs=4) as sm, \
             tc.tile_pool(name="ps_s", bufs=2, space="PSUM") as ps_s, \
             tc.tile_pool(name="ps_o", bufs=2, space="PSUM") as ps_o, \
             tc.tile_pool(name="ps_d", bufs=1, space="PSUM") as ps_d, \
             tc.tile_pool(name="ps_t", bufs=1, space="PSUM") as ps_t:

            # ---- constants ----
            utm = consts.tile([P, P], BF16)  # utm[k,q] = 1 iff q >= k
            make_upper_triangular(nc, utm, val=1.0, diag=True)
            ident_h = consts.tile([P, P], BF16)
            make_identity(nc, ident_h)
            ones_col = consts.tile([P, 1], BF16)
            nc.vector.memset(ones_col, 1.0)
            # fp8 ones for DoubleRow den matmuls; the pair dim is sliced out
            # of a padded tile so its byte-step is 16 (ISA requirement)
            ones8p = consts.tile([P, 2, 16], FP8)
            nc.vector.memset(ones8p, 1.0)
            nbias = consts.tile([P, 1], F32)
            nc.vector.memset(nbias, -2.0)

            # ---- PE clock warmup: dummy bf16 matmuls while DMAs land ----
            for w in range(N_WARMUP):
                pw = ps_o.tile([P, CHUNK], F32, tag="po", name="po")
                nc.tensor.matmul(pw[:, 0:P], utm, utm, start=True, stop=True,
                                 skip_group_check=True)

            head_state = {}

            def emit_load(hh, first=False):
                h = hh % HEADS_PER_CORE
                qT = qkt.tile([P, S], BF16, tag="qT", name="qT")
                kT = qkt.tile([P, S], BF16, tag="kT", name="kT")
                vnr = qkt.tile([P, N_KT, P], BF16, tag="vnr", name="vnr")
                v8 = qkt.tile([P, N_KT, P], FP8, tag="v8", name="v8")
                st = dict(qT=qT, kT=kT, vnr=vnr, v8=v8)
                head_state[hh] = st
                if first:
                    # k first (first matmuls need it), in halves so the first
                    # chunk can start after ~1 us of DMA
                    h8 = N_KT // 2
                    nc.sync.dma_start(out=kT[:, 0:S // 2],
                                      in_=kt_d[h, :, 0:S // 2])
                    nc.sync.dma_start(out=qT[:, 0:S // 2],
                                      in_=qt_d[h, :, 0:S // 2])
                    nc.sync.dma_start(
                        out=vnr[:, 0:h8],
                        in_=v_d[h, 0:S // 2].rearrange("(t p) d -> p t d", p=P))
                    nc.sync.dma_start(
                        out=st["v8"][:, 0:h8],
                        in_=v8_d[h, 0:S // 2].rearrange("(t p) d -> p t d", p=P))
                    nc.sync.dma_start(out=kT[:, S // 2:S],
                                      in_=kt_d[h, :, S // 2:S])
                    nc.sync.dma_start(out=qT[:, S // 2:S],
                                      in_=qt_d[h, :, S // 2:S])
                    nc.sync.dma_start(
                        out=vnr[:, h8:N_KT],
                        in_=v_d[h, S // 2:S].rearrange("(t p) d -> p t d", p=P))
                    nc.sync.dma_start(
                        out=st["v8"][:, h8:N_KT],
                        in_=v8_d[h, S // 2:S].rearrange("(t p) d -> p t d", p=P))
                else:
                    nc.sync.dma_start(out=kT, in_=kt_d[h])
                    nc.sync.dma_start(out=qT, in_=qt_d[h])
                    nc.sync.dma_start(
                        out=vnr, in_=v_d[h].rearrange("(t p) d -> p t d", p=P))
                    nc.sync.dma_start(
                        out=st["v8"], in_=v8_d[h].rearrange("(t p) d -> p t d", p=P))

            def make_pv(st, offs, pexp, psum_o, psum_d, jmax):
                def emit():
                    for (j, oj, base) in offs:
                        nc.tensor.matmul(
                            psum_o[:, oj:CHUNK], st["vnr"][:, j, :],
                            pexp[:, base + oj:base + CHUNK],
                            start=(j == 0), stop=(j == jmax),
                            skip_group_check=True)
                        nc.tensor.matmul(
                            psum_d[:, oj:CHUNK], ones_col,
                            pexp[:, base + oj:base + CHUNK],
                            start=(j == 0), stop=(j == jmax),
                            skip_group_check=True)
                return emit

            def make_pv_dr(st, js, pexp8, psum_o, psum_d, jmax):
                # fp8 DoubleRow: one matmul contracts both key tiles of the
                # pair (256 rows) at 0.5 cycles/row
                j0 = js[0]
                def emit():
                    pe3 = pexp8.rearrange("p (i n) -> p i n", i=2)
                    nc.tensor.matmul(
                        psum_o, st["v8"][:, j0:j0 + 2, :], pe3,
                        start=(j0 == 0), stop=(j0 + 1 == jmax),
                        perf_mode=mybir.MatmulPerfMode.DoubleRow,
                        skip_group_check=True)
                    nc.tensor.matmul(
                        psum_d, ones8p[:, :, 0:1], pe3,
                        start=(j0 == 0), stop=(j0 + 1 == jmax),
                        perf_mode=mybir.MatmulPerfMode.DoubleRow,
                        skip_group_check=True)
                return emit

            def make_tail(hh, c, psum_o, psum_d):
                def emit():
                    h = hh % HEADS_PER_CORE
                    # denominators out of PSUM first (releases the den bank),
                    # then reciprocal once per chunk
                    dens = sm.tile([1, CHUNK], F32, tag="dens", name="dens")
                    nc.vector.tensor_copy(dens, psum_d)
                    rcq = sm.tile([1, CHUNK], F32, tag="rcq", name="rcq")
                    nc.vector.reciprocal_approx_fast(rcq, dens)
                    # physically replicate 1/den across partitions (gpsimd),
                    # then normalize OUT^T columns with one elementwise mul
                    rcb = sm.tile([P, CHUNK], F32, tag="rcb", name="rcb")
                    nc.gpsimd.partition_broadcast(rcb, rcq)
                    outn = sm.tile([P, CHUNK], BF16, tag="outn", name="outn")
                    nc.vector.tensor_mul(outn, psum_o, rcb)
                    # transpose OUT^T back to [q, d] (bf16, cheap)
                    ptr2 = ps_t.tile([P, CHUNK], BF16, tag="ptr", name="ptr")
                    for tt in range(4):
                        nc.tensor.transpose(
                            ptr2[:, tt * P:(tt + 1) * P],
                            outn[:, tt * P:(tt + 1) * P], ident_h)
                    # evacuate + widen to f32 for the store
                    outT = sm.tile([P, 4, P], F32, tag="outT", name="outT")
                    nc.vector.tensor_copy(
                        outT, ptr2.rearrange("p (a b) -> p a b", b=P))
                    nc.sync.dma_start(
                        out=o_d[h, CHUNK * c:CHUNK * (c + 1), :].rearrange(
                            "(t p) d -> p t d", p=P),
                        in_=outT)
                return emit

            emit_load(0, first=True)

            for hh in range(n_heads):
                st = head_state[hh]
                if hh + 1 < n_heads:
                    emit_load(hh + 1)

                pending_pv = None          # PV/den of previous group
                deferred = []              # [(age, closure)] chunk tails
                group_idx = 0

                for c in range(N_CH):
                    jmax = 4 * c + 3
                    psum_o = ps_o.tile([P, CHUNK], F32, tag="po", name="po")
                    psum_d = ps_d.tile([1, CHUNK], F32, tag="pd", name="pd")

                    for jp in range((jmax + 2) // 2):
                        j0 = 2 * jp
                        js = [j for j in (j0, j0 + 1) if j <= jmax]
                        psum_s = ps_s.tile([P, 2 * CHUNK], F32, tag="psm",
                                           name="psm")
                        nondiag = (j0 + 1 <= jmax) and ((j0 + 1) * P < CHUNK * c)
                        pexp = px.tile([P, 2 * CHUNK], FP8 if nondiag else BF16,
                                       tag="pexp", name="pexp")

                        offs = []
                        for j in js:
                            oj = max(0, P * j - CHUNK * c)
                            base = (j - j0) * CHUNK
                            offs.append((j, oj, base))
                            # full-width write even on diagonal tiles: the
                            # below-diagonal columns hold bounded garbage that
                            # exp processes but PV/den never consume; this
                            # keeps every exp read covered by this tile's
                            # writes (no stale-PSUM reads)
                            nc.tensor.matmul(
                                psum_s[:, base:base + CHUNK],
                                st["kT"][:, j * P:(j + 1) * P],
                                st["qT"][:, CHUNK * c:CHUNK * (c + 1)],
                                start=True, stop=True)

                        # exp (+ causal masking of diagonal 128-blocks,
                        # applied in place after the exp). Diagonal pairs are
                        # exp'd in a single wide op spanning both j regions;
                        # the dead gap between them is never consumed.
                        diag = any(j * P >= CHUNK * c for (j, oj, base) in offs)
                        lo = offs[0][2] + offs[0][1]
                        hi = offs[-1][2] + CHUNK
                        # uniform logit shift keeps exp inside fp8e4m3
                        # range (softmax is shift-invariant; den and out
                        # scale by the same e^-2)
                        nc.scalar.activation(
                            pexp[:, lo:hi], psum_s[:, lo:hi],
                            EXP, scale=1.0 / TEMPERATURE, bias=nbias)
                        if diag:
                            for (j, oj, base) in offs:
                                if j * P >= CHUNK * c:
                                    nc.vector.tensor_mul(
                                        pexp[:, base + oj:base + oj + P],
                                        pexp[:, base + oj:base + oj + P], utm)

                        if pending_pv is not None:
                            pending_pv()
                        if nondiag:
                            pending_pv = make_pv_dr(st, js, pexp, psum_o,
                                                    psum_d, jmax)
                        else:
                            pending_pv = make_pv(st, offs, pexp, psum_o,
                                                 psum_d, jmax)

                        group_idx += 1
                        tail_age = 1 if hh == n_heads - 1 else 2
                        for item in list(deferred):
                            if group_idx - item[0] >= tail_age:
                                item[1]()
                                deferred.remove(item)

                    deferred.append((group_idx, make_tail(hh, c, psum_o,
                                                          psum_d)))

                # flush this head
                if pending_pv is not None:
                    pending_pv()
                for item in deferred:
                    item[1]()

    nc.compile()
    return nc


_NC_CACHE = None


def _get_nc():
    global _NC_CACHE
    if _NC_CACHE is None:
        _NC_CACHE = build_attention_nc()
    return _NC_CACHE


def kernel(q, k, v, mask=None, _trace=False):
    """Full-input entry point: q,k,v [2,16,2048,128] f32, mask [2,1,2048,2048]
    int32 (causal; the kernel hardcodes causality and does not read it).
    Returns [2,16,2048,128] f32. Q/K are transposed and all inputs cast to
    bf16 host-side as part of the sharding step."""
    nc = _get_nc()
    BH = B * H
    qt = np.ascontiguousarray(
        np.asarray(q, dtype=np.float32).reshape(BH, S, D).transpose(0, 2, 1)
    ).astype(ml_dtypes.bfloat16)
    kt = np.ascontiguousarray(
        np.asarray(k, dtype=np.float32).reshape(BH, S, D).transpose(0, 2, 1)
    ).astype(ml_dtypes.bfloat16)
    vf32 = np.asarray(v, dtype=np.float32).reshape(BH, S, D)
    vf = vf32.astype(ml_dtypes.bfloat16)
    v8 = vf32.astype(mybir.dt.np(mybir.dt.float8e4))
    in_maps = []
    for i in range(N_CORES):
        sl = slice(i * HEADS_PER_CORE, (i + 1) * HEADS_PER_CORE)
        in_maps.append({"qt": qt[sl], "kt": kt[sl], "v": vf[sl],
                        "v8": v8[sl]})
    res = run_bass_kernel_spmd(nc, in_maps, list(range(N_CORES)), trace=_trace)
    out = np.concatenate([res.results[i]["out"] for i in range(N_CORES)], axis=0)
    out = out.reshape(B, H, S, D).astype(np.float32)
    if _trace:
        return out, res
    return out
